# revision 49
# baseline (speedup 1.0000x reference)
"""LRU forward on 8 Trainium2 NeuronCores.

Sharding: 8 shards = 4 batches x 2 sequence halves (L_local = 2048).
Per-core dataflow is fully transposed (d_model on SBUF partitions, time on
the free dim):

  input proj   Bu^T = Bg_cat^T @ x^T as fp8e4 DoubleRow matmuls (weights
               pre-scaled x64 on the host to stay in e4m3 normal range; the
               1/64 is folded into the PSUM->SBUF downcast scale)
  scan         complex diagonal recurrence -> rotating frame e^{-i theta t}
               turns it into 4 real per-lane scans (hardware
               tensor_tensor_scan; fp32 multiplier + internal state, fp16
               data).  Carry between sequence halves exchanged with a
               pairwise AllReduce and applied as g += r^{512m+s+1} * c_hat
               (decay table has the per-chunk factor folded in); chunk 0's
               rot-out runs carry-free inside the exchange bubble and is
               corrected afterwards via host P/Q tables (rpow x cos/sin)
  output proj  ys^T = CT_cat^T @ h in fp16, plus the D*x skip path as fp8
               DoubleRow matmuls over hi/lo split planes of x against
               duplicated diag(D_hi)/diag(D_lo) blocks (exact to ~0.4%)

Rotation elementwise work runs on DVE in fp16 (2x perf mode) with the c=1
imaginary path offloaded to gpsimd; PSUM->SBUF downcasts run on the scalar
(Activation) engine; tables are SBUF resident and loaded once.  Phase A uses
non-uniform time chunks (small at both ends) to shorten pipeline fill and
the scan->collective tail, and junk matmuls chained on the exchange arrival
keep the tensor engine's p-state ramped through the collective bubble.
Host side only preprocesses/shards and reassembles the output.
"""

import os

import numpy as np

B, L, D, N = 4, 4096, 1024, 256
NCORE = 8
LLOC = L // 2          # per-core sequence length
MC = 512               # time chunk (matmul moving free dim)
NMC = LLOC // MC       # 4 chunks
N2 = 2 * N             # stacked re|im channels

_CACHE = {}
LAST_RESULTS = None    # test.py reads exec_time_ns from here


def _build():
    import concourse.bass as bass
    import concourse.mybir as mybir
    import concourse.tile as tile
    from concourse import bacc

    f32 = mybir.dt.float32
    f16 = mybir.dt.float16
    f8 = mybir.dt.float8e4
    DR = mybir.MatmulPerfMode.DoubleRow
    ADD = mybir.AluOpType.add
    SUB = mybir.AluOpType.subtract
    MUL = mybir.AluOpType.mult

    nc = bacc.Bacc("TRN2", target_bir_lowering=False, debug=False, num_devices=NCORE)

    # ---- DRAM I/O (per-core) ----
    # xq holds fp8 hi/lo split planes of x^T: [:, 0] = fp8(x), [:, 1] =
    # fp8(x - hi).  The input projection reads the hi plane; the skip path
    # contracts both planes against duplicated diag(D_hi)/diag(D_lo) blocks.
    xqd = nc.dram_tensor("xq", [128, 2, 8, LLOC], f8, kind="ExternalInput").ap()
    bg8d = nc.dram_tensor("bg8", [128, 8, N2], f8, kind="ExternalInput").ap()
    ctd = nc.dram_tensor("ct", [128, 4, D], f16, kind="ExternalInput").ap()
    ddd = nc.dram_tensor("ddiag", [128, 2, 8, 2, 128], f8, kind="ExternalInput").ap()
    csd = nc.dram_tensor("cost", [128, 2, LLOC], f16, kind="ExternalInput").ap()
    snd = nc.dram_tensor("sint", [128, 2, LLOC], f16, kind="ExternalInput").ap()
    rbd = nc.dram_tensor("rb", [128, 2, MC], f32, kind="ExternalInput").ap()
    rpwd = nc.dram_tensor("rpow", [128, 2, NMC, MC], f16, kind="ExternalInput").ap()
    pcd = nc.dram_tensor("pctab", [128, 2, MC], f16, kind="ExternalInput").ap()
    qcd = nc.dram_tensor("qctab", [128, 2, MC], f16, kind="ExternalInput").ap()
    r48d = nc.dram_tensor("rot48", [128, 2, 3], f32, kind="ExternalInput").ap()
    gmd = nc.dram_tensor("gmask", [128, 4], f32, kind="ExternalInput").ap()
    pmd = nc.dram_tensor("pmask", [128, 4], f32, kind="ExternalInput").ap()
    outd = nc.dram_tensor("outT", [128, 8, LLOC], f16, kind="ExternalOutput").ap()

    with tile.TileContext(nc) as tc:
        from contextlib import ExitStack

        with ExitStack() as st:
            cpool = st.enter_context(tc.tile_pool(name="consts", bufs=1))
            xpool = st.enter_context(tc.tile_pool(name="xt", bufs=1))
            gpool = st.enter_context(tc.tile_pool(name="g", bufs=1))
            bpool = st.enter_context(tc.tile_pool(name="bu", bufs=3))
            upool = st.enter_context(tc.tile_pool(name="u", bufs=3))
            hpool = st.enter_context(tc.tile_pool(name="h", bufs=3))
            opool = st.enter_context(tc.tile_pool(name="o", bufs=3))
            ps = st.enter_context(tc.tile_pool(name="ps", bufs=2, space="PSUM"))
            dram = st.enter_context(tc.tile_pool(name="dram", bufs=1, space="DRAM"))

            # ---- SBUF residents; DMA order is pipeline priority order ----
            bg8_sb = cpool.tile([128, 8, N2], f8, tag="bg8", name="bg8")
            xq_sb = xpool.tile([128, 2, 8, LLOC], f8, tag="xq", name="xq")
            cs_sb = cpool.tile([128, 2, LLOC], f16, tag="cs", name="cs")
            sn_sb = cpool.tile([128, 2, LLOC], f16, tag="sn", name="sn")
            rb_sb = cpool.tile([128, 2, MC], f32, tag="rb", name="rb")
            ct_sb = cpool.tile([128, 4, D], f16, tag="ct", name="ct")
            dd_sb = cpool.tile([128, 2, 8, 2, 128], f8, tag="dd", name="dd")
            rpw_sb = cpool.tile([128, 2, NMC, MC], f16, tag="rpw", name="rpw")
            pc_sb = cpool.tile([128, 2, MC], f16, tag="pc", name="pc")
            qc_sb = cpool.tile([128, 2, MC], f16, tag="qc", name="qc")
            r48_sb = cpool.tile([128, 2, 3], f32, tag="r48", name="r48")
            gm_sb = cpool.tile([128, 4], f32, tag="gm", name="gm")
            pm_sb = cpool.tile([128, 4], f32, tag="pm", name="pm")

            # phase-A chunking: small chunks at the ends to shorten pipeline
            # fill and the scan->collective tail latency
            CHA = [(0, 256), (256, 256), (512, 512), (1024, 512),
                   (1536, 384), (1920, 128)]

            nc.sync.dma_start(bg8_sb[:, 0:4, :], bg8d[:, 0:4, :])
            nc.sync.dma_start(xq_sb[:, 0, :, 0:256], xqd[:, 0, :, 0:256])
            nc.sync.dma_start(bg8_sb[:, 4:8, :], bg8d[:, 4:8, :])
            nc.sync.dma_start(xq_sb[:, 0, :, 256:MC], xqd[:, 0, :, 256:MC])
            nc.sync.dma_start(cs_sb[:, :, 0:2 * MC], csd[:, :, 0:2 * MC])
            nc.sync.dma_start(sn_sb[:, :, 0:2 * MC], snd[:, :, 0:2 * MC])
            nc.sync.dma_start(rb_sb[:], rbd[:, :, :])
            nc.sync.dma_start(xq_sb[:, 0, :, MC:2 * MC], xqd[:, 0, :, MC:2 * MC])
            nc.sync.dma_start(cs_sb[:, :, 2 * MC:], csd[:, :, 2 * MC:])
            nc.sync.dma_start(sn_sb[:, :, 2 * MC:], snd[:, :, 2 * MC:])
            nc.sync.dma_start(xq_sb[:, 0, :, 2 * MC:3 * MC],
                              xqd[:, 0, :, 2 * MC:3 * MC])
            nc.sync.dma_start(xq_sb[:, 0, :, 3 * MC:], xqd[:, 0, :, 3 * MC:])
            nc.sync.dma_start(ct_sb[:], ctd[:, :, :])
            nc.sync.dma_start(dd_sb[:], ddd[:, :, :, :, :])
            nc.sync.dma_start(rpw_sb[:], rpwd[:, :, :, :])
            nc.sync.dma_start(pc_sb[:], pcd[:, :, :])
            nc.sync.dma_start(qc_sb[:], qcd[:, :, :])
            nc.sync.dma_start(r48_sb[:], r48d[:, :, :])
            nc.sync.dma_start(gm_sb[:], gmd[:, :])
            nc.sync.dma_start(pm_sb[:], pmd[:, :])
            # x lo-plane loads late: its consumers (skip-path matmuls) run
            # inside the collective bubble, and an early load would displace
            # phase-A-critical transfers on the DMA engines
            nc.sync.dma_start(xq_sb[:, 1, :, :], xqd[:, 1, :, :])

            g4 = []
            for tt_ in range(4):
                g4.append(gpool.tile([128, LLOC], f16, tag=f"g{tt_}", name=f"g{tt_}"))


            # ---- phase A: input projection + rot-in + scan, per time chunk ----
            # input proj: fp8 DoubleRow matmuls (weights pre-scaled x64 on the
            # host; the 1/64 is folded into the PSUM->SBUF downcast scale).
            # rot-in: the c=1 imaginary path runs on gpsimd to unload DVE.
            for m, (st, w) in enumerate(CHA):
                ms = slice(st, st + w)
                bu = []
                for j in range(4):
                    pt = ps.tile([128, MC], f32, tag=f"p{j}", name=f"pbu{j}_{m}")
                    for k2 in range(4):
                        nc.tensor.matmul(
                            pt[:, 0:w],
                            bg8_sb[:, 2 * k2:2 * k2 + 2, 128 * j:128 * (j + 1)],
                            xq_sb[:, 0, 2 * k2:2 * k2 + 2, ms],
                            start=(k2 == 0),
                            stop=(k2 == 3),
                            perf_mode=DR,
                        )
                    bt = bpool.tile([128, MC], f16, tag=f"bu{j}", name=f"bu{j}_{m}")
                    nc.scalar.mul(bt[:, 0:w], pt[:, 0:w], 1.0 / 64.0)
                    bu.append(bt)
                # rot-in, ordered by bu-copy availability (bu0, bu1, bu2, bu3
                # land in sequence from the Activation queue): ops consuming
                # early copies are emitted first so DVE never waits on a
                # later copy while an earlier one is ready
                u4 = [None] * 4
                cs0, sn0 = cs_sb[:, 0, ms], sn_sb[:, 0, ms]
                cs1, sn1 = cs_sb[:, 1, ms], sn_sb[:, 1, ms]
                t1a = upool.tile([128, MC], f16, tag="tmpA", name=f"tA0_{m}")
                nc.vector.tensor_tensor(t1a[:, 0:w], bu[0][:, 0:w], cs0, MUL)
                t4a = upool.tile([128, MC], f16, tag="tmpD0", name=f"tD0_{m}")
                nc.vector.tensor_tensor(t4a[:, 0:w], bu[0][:, 0:w], sn0, MUL)
                t1b = upool.tile([128, MC], f16, tag="tmpB0", name=f"tB0_{m}")
                nc.vector.tensor_tensor(t1b[:, 0:w], bu[1][:, 0:w], cs1, MUL)
                t4b = upool.tile([128, MC], f16, tag="tmpD1", name=f"tD1_{m}")
                nc.gpsimd.tensor_tensor(t4b[:, 0:w], bu[1][:, 0:w], sn1, MUL)
                t2a = upool.tile([128, MC], f16, tag="tmpC0", name=f"tC0_{m}")
                nc.vector.tensor_tensor(t2a[:, 0:w], bu[2][:, 0:w], sn0, MUL)
                t3a = upool.tile([128, MC], f16, tag="tmpE0", name=f"tE0_{m}")
                nc.vector.tensor_tensor(t3a[:, 0:w], bu[2][:, 0:w], cs0, MUL)
                u_re0 = upool.tile([128, MC], f16, tag="u0", name=f"u0_{m}")
                nc.vector.tensor_tensor(u_re0[:, 0:w], t1a[:, 0:w], t2a[:, 0:w], ADD)
                u_im0 = upool.tile([128, MC], f16, tag="u2", name=f"u2_{m}")
                nc.vector.tensor_tensor(u_im0[:, 0:w], t3a[:, 0:w], t4a[:, 0:w], SUB)
                u4[0], u4[2] = u_re0, u_im0
                t2b = upool.tile([128, MC], f16, tag="tmpC1", name=f"tC1_{m}")
                nc.vector.tensor_tensor(t2b[:, 0:w], bu[3][:, 0:w], sn1, MUL)
                t3b = upool.tile([128, MC], f16, tag="tmpE1", name=f"tE1_{m}")
                nc.gpsimd.tensor_tensor(t3b[:, 0:w], bu[3][:, 0:w], cs1, MUL)
                u_re1 = upool.tile([128, MC], f16, tag="u1", name=f"u1_{m}")
                nc.vector.tensor_tensor(u_re1[:, 0:w], t1b[:, 0:w], t2b[:, 0:w], ADD)
                u_im1 = upool.tile([128, MC], f16, tag="u3", name=f"u3_{m}")
                nc.gpsimd.tensor_tensor(u_im1[:, 0:w], t3b[:, 0:w], t4b[:, 0:w], SUB)
                u4[1], u4[3] = u_re1, u_im1
                for tt_ in range(4):
                    init = 0.0 if m == 0 else g4[tt_][:, st - 1:st]
                    nc.vector.tensor_tensor_scan(
                        g4[tt_][:, ms], rb_sb[:, tt_ & 1, 0:w], u4[tt_][:, 0:w],
                        init, MUL, ADD)

            # ---- phase B: carry exchange (pairwise AllReduce) ----
            stage = cpool.tile([128, 4], f32, tag="stage", name="stage")
            for tt_ in range(4):
                nc.vector.tensor_copy(stage[:, tt_:tt_ + 1], g4[tt_][:, LLOC - 1:LLOC])
            # scatter my carry into my pair's 4-column group (zero elsewhere)
            stage16 = cpool.tile([128, 16], f32, tag="stage16", name="stage16")
            for p in range(4):
                nc.vector.tensor_scalar_mul(
                    stage16[:, 4 * p:4 * (p + 1)], stage[:], gm_sb[:, p:p + 1])
            in_cc = dram.tile([128, 16], f32, tag="incc", name="incc")
            out_cc = dram.tile([128, 16], f32, tag="outcc", name="outcc",
                               addr_space="Shared")
            nc.sync.dma_start(in_cc[:], stage16[:])
            if os.environ.get("LRU_NOCC", "0") == "1":
                # collective-free variant for TimelineSim bottleneck analysis
                nc.sync.dma_start(out_cc[:], in_cc[:])
            else:
                nc.gpsimd.collective_compute(
                    "AllReduce",
                    mybir.AluOpType.add,
                    replica_groups=[list(range(NCORE))],
                    ins=[in_cc.opt()],
                    outs=[out_cc.opt()],
                )
            recv16 = cpool.tile([128, 16], f32, tag="recv16", name="recv16")
            nc.sync.dma_start(recv16[:], out_cc[:])


            # skip-path matmuls for chunk 0 run inside the collective bubble
            m0 = slice(0, MC)
            pre_ps = []
            with tc.high_priority():
                for di in range(8):
                    pt = ps.tile([128, MC], f32, tag=f"p{di % 4}", name=f"o{di}_0")
                    if di > 0:
                        for i in range(2):
                            nc.tensor.matmul(pt[:], dd_sb[:, i, di, :, :],
                                             xq_sb[:, :, di, m0],
                                             start=(i == 0), stop=False,
                                             perf_mode=DR)
                    pre_ps.append(pt)

            # PE warm-up chained on the exchange arrival: junk matmuls into a
            # corner of pre_ps[0] (its real skip matmul below uses start=True
            # and overwrites) keep the tensor engine's p-state ramped through
            # the bubble so the first output matmuls run at full clock
            jsrc = cpool.tile([128, 16], f16, tag="jsrc", name="jsrc")
            nc.scalar.copy(jsrc[:], recv16[:])
            for _ in range(60):
                nc.tensor.matmul(pre_ps[0][0:16, 0:128], jsrc[:, 0:16],
                                 cs_sb[:, 0, 0:128], start=True, stop=True)
            for i in range(2):
                nc.tensor.matmul(pre_ps[0][:], dd_sb[:, i, 0, :, :],
                                 xq_sb[:, :, 0, m0],
                                 start=(i == 0), stop=False, perf_mode=DR)

            # chunk-0 carry-free rot-out also runs inside the bubble; the
            # carry correction lands later via the P/Q tables (rpow folded
            # with cos/sin on the host) and per-lane chat scalars.
            # wait-hint: schedule these AFTER the exchange staging ops above
            # (their data deps are ready much earlier and the scheduler would
            # otherwise run them first, delaying the exchange).
            tc.tile_set_cur_wait(0.040)
            h4_0 = [None] * 4
            for c in range(2):
                csm = cs_sb[:, c, m0]
                snm = sn_sb[:, c, m0]
                t1 = hpool.tile([128, MC], f16, tag="tmpC", name=f"tE{c}_b")
                nc.vector.tensor_tensor(t1[:], g4[c][:, m0], csm, MUL)
                t2 = hpool.tile([128, MC], f16, tag="tmpD", name=f"tF{c}_b")
                nc.vector.tensor_tensor(t2[:], g4[2 + c][:, m0], snm, MUL)
                h_re = hpool.tile([128, MC], f16, tag=f"h{c}", name=f"h{c}_b")
                nc.vector.tensor_tensor(h_re[:], t1[:], t2[:], SUB)
                h4_0[c] = h_re
                t3 = hpool.tile([128, MC], f16, tag="tmpC", name=f"tG{c}_b")
                nc.vector.tensor_tensor(t3[:], g4[2 + c][:, m0], csm, MUL)
                t4 = hpool.tile([128, MC], f16, tag="tmpD", name=f"tH{c}_b")
                nc.vector.tensor_tensor(t4[:], g4[c][:, m0], snm, MUL)
                h_im = hpool.tile([128, MC], f16, tag=f"h{2+c}", name=f"h{2+c}_b")
                nc.vector.tensor_tensor(h_im[:], t3[:], t4[:], ADD)
                h4_0[2 + c] = h_im

            # select my pair's group: recv = sum_p recv16[:, 4p:4p+4] * pm[:, p]
            recv = cpool.tile([128, 4], f32, tag="recv", name="recv")
            nc.vector.tensor_scalar_mul(recv[:], recv16[:, 0:4], pm_sb[:, 0:1])
            for p in range(1, 4):
                nc.vector.scalar_tensor_tensor(
                    recv[:], recv16[:, 4 * p:4 * (p + 1)], pm_sb[:, p:p + 1],
                    recv[:], MUL, ADD)

            # chat = rot48 * recv (per-lane complex rotation), via STT pairs;
            # cols 4,5 hold -chat_im for the chunk-0 P/Q correction
            chat = cpool.tile([128, 6], f32, tag="chat", name="chat")
            tca = cpool.tile([128, 1], f32, tag="tca", name="tca")
            tcb = cpool.tile([128, 1], f32, tag="tcb", name="tcb")
            for c in range(2):
                c48 = r48_sb[:, c, 0:1]
                s48 = r48_sb[:, c, 1:2]
                ns48 = r48_sb[:, c, 2:3]
                # chat_re = recv_re*cos48 - recv_im*sin48
                nc.vector.tensor_tensor(tca[:], recv[:, c:c + 1], c48, MUL)
                nc.vector.scalar_tensor_tensor(
                    chat[:, c:c + 1], recv[:, 2 + c:3 + c], ns48, tca[:], MUL, ADD)
                # chat_im = recv_im*cos48 + recv_re*sin48
                nc.vector.tensor_tensor(tcb[:], recv[:, 2 + c:3 + c], c48, MUL)
                nc.vector.scalar_tensor_tensor(
                    chat[:, 2 + c:3 + c], recv[:, c:c + 1], s48, tcb[:], MUL, ADD)
                nc.vector.tensor_scalar_mul(
                    chat[:, 4 + c:5 + c], chat[:, 2 + c:3 + c], -1.0)

            # ---- phase C: carry fix + rot-out + output projection ----
            # h4 index: 0 = re(c0), 1 = re(c1), 2 = im(c0), 3 = im(c1); the
            # output matmuls consume them in that order, so compute the re
            # components (and their carry fixes) first.
            # tile_wait_until: scheduler-only hint that phase C becomes ready
            # late (after the carry exchange) so ready-early work (the
            # skip-path matmuls) is ordered ahead of it in the engine streams.
            tc.tile_set_cur_wait(0.045)
            for m in range(NMC):
                ms = slice(m * MC, (m + 1) * MC)

                if m == 0:
                    # apply the carry to the pre-computed local rot-out:
                    # h_re += chat_re*P - chat_im*Q ; h_im += chat_im*P + chat_re*Q
                    h4 = h4_0
                    for comp, pcol, qcol in ((0, 0, 4), (1, 1, 5),
                                             (2, 2, 0), (3, 3, 1)):
                        c = comp & 1
                        nc.vector.scalar_tensor_tensor(
                            h4[comp][:], pc_sb[:, c, :], chat[:, pcol:pcol + 1],
                            h4[comp][:], MUL, ADD)
                        nc.vector.scalar_tensor_tensor(
                            h4[comp][:], qc_sb[:, c, :], chat[:, qcol:qcol + 1],
                            h4[comp][:], MUL, ADD)
                else:
                    # carry fix (STT only runs on DVE; the compiler rejects
                    # it on the gpsimd engine)
                    for tt_ in (0, 2, 1, 3):
                        nc.vector.scalar_tensor_tensor(
                            g4[tt_][:, ms],
                            rpw_sb[:, tt_ & 1, m, :],
                            chat[:, tt_:tt_ + 1],
                            g4[tt_][:, ms],
                            MUL,
                            ADD,
                        )
                    h4 = [None] * 4
                    for c in range(2):
                        csm = cs_sb[:, c, ms]
                        snm = sn_sb[:, c, ms]
                        t1 = hpool.tile([128, MC], f16, tag="tmpC", name=f"tE{c}_{m}")
                        nc.vector.tensor_tensor(t1[:], g4[c][:, ms], csm, MUL)
                        t2 = hpool.tile([128, MC], f16, tag="tmpD", name=f"tF{c}_{m}")
                        nc.vector.tensor_tensor(t2[:], g4[2 + c][:, ms], snm, MUL)
                        h_re = hpool.tile([128, MC], f16, tag=f"h{c}", name=f"h{c}_{m}")
                        nc.vector.tensor_tensor(h_re[:], t1[:], t2[:], SUB)
                        h4[c] = h_re
                    for c in range(2):
                        csm = cs_sb[:, c, ms]
                        snm = sn_sb[:, c, ms]
                        t3 = hpool.tile([128, MC], f16, tag="tmpC", name=f"tG{c}_{m}")
                        nc.vector.tensor_tensor(t3[:], g4[2 + c][:, ms], csm, MUL)
                        t4 = hpool.tile([128, MC], f16, tag="tmpD", name=f"tH{c}_{m}")
                        nc.vector.tensor_tensor(t4[:], g4[c][:, ms], snm, MUL)
                        h_im = hpool.tile([128, MC], f16, tag=f"h{2+c}",
                                          name=f"h{2+c}_{m}")
                        nc.vector.tensor_tensor(h_im[:], t3[:], t4[:], ADD)
                        h4[2 + c] = h_im
                ot = opool.tile([128, 8, MC], f16, tag="ot", name=f"ot_{m}")
                for di in range(8):
                    if m == 0:
                        pt = pre_ps[di]
                    else:
                        pt = ps.tile([128, MC], f32, tag=f"p{di % 4}",
                                     name=f"o{di}_{m}")
                        for i in range(2):
                            nc.tensor.matmul(pt[:], dd_sb[:, i, di, :, :],
                                             xq_sb[:, :, di, ms],
                                             start=(i == 0), stop=False,
                                             perf_mode=DR)
                    for tt_ in range(4):
                        nc.tensor.matmul(
                            pt[:],
                            ct_sb[:, tt_, 128 * di:128 * (di + 1)],
                            h4[tt_][:],
                            start=False,
                            stop=(tt_ == 3),
                        )
                    nc.scalar.copy(ot[:, di, :], pt[:])
                    if di == 3:
                        nc.sync.dma_start(outd[:, 0:4, ms], ot[:, 0:4, :])
                    elif m == NMC - 1 and di in (5, 6):
                        nc.sync.dma_start(outd[:, di - 1:di, ms],
                                          ot[:, di - 1:di, :])
                if m == NMC - 1:
                    nc.sync.dma_start(outd[:, 6:7, ms], ot[:, 6:7, :])
                    nc.sync.dma_start(outd[:, 7:8, ms], ot[:, 7:8, :])
                else:
                    nc.sync.dma_start(outd[:, 4:8, ms], ot[:, 4:8, :])

    nc.compile()
    return nc


def _prep(inputs):
    """Host-side parameter prep + sharding. Returns per-core input maps."""
    x = np.asarray(inputs["input_sequence"], np.float32)
    nu_log = np.asarray(inputs["nu_log"], np.float32)
    theta_log = np.asarray(inputs["theta_log"], np.float32)
    B_re = np.asarray(inputs["B_re"], np.float32)
    B_im = np.asarray(inputs["B_im"], np.float32)
    C_re = np.asarray(inputs["C_re"], np.float32)
    C_im = np.asarray(inputs["C_im"], np.float32)
    Dv = np.asarray(inputs["D"], np.float32)

    r32 = np.exp(-np.exp(nu_log, dtype=np.float32), dtype=np.float32)
    th = np.exp(theta_log, dtype=np.float32).astype(np.float64)
    gamma = np.sqrt((1.0 - r32 * r32).astype(np.float32))

    def blk(a, nb):  # [nb*128, F] -> [128, nb, F]
        return np.ascontiguousarray(
            a.reshape(nb, 128, a.shape[-1]).transpose(1, 0, 2))

    import ml_dtypes
    f8 = ml_dtypes.float8_e4m3

    bg = np.concatenate(
        [(gamma[:, None] * B_re).T, (gamma[:, None] * B_im).T], axis=1)  # [D, 512]
    ct = np.concatenate([C_re.T, -C_im.T], axis=0)                      # [512, D]
    # fp8 weights pre-scaled x64 to stay in e4m3 normal range; the matmul
    # consumer divides by 64 during the PSUM downcast
    bg83 = blk(bg * 64.0, 8).astype(f8)
    ct3 = blk(ct, 4).astype(np.float16)

    t = np.arange(LLOC, dtype=np.float64)
    ang = th[:, None] * t[None, :]
    cs3 = blk(np.cos(ang), 2).astype(np.float16)
    sn3 = blk(np.sin(ang), 2).astype(np.float16)
    r64 = r32.astype(np.float64)
    s = np.arange(MC, dtype=np.float64)
    # r^(512m + s + 1), chunk factor folded in
    rpw = (r64[:, None, None]
           ** (MC * np.arange(NMC, dtype=np.float64)[None, :, None]
               + s[None, None, :] + 1.0))                     # [N, NMC, MC]
    rpw3 = np.ascontiguousarray(
        rpw.reshape(2, 128, NMC, MC).transpose(1, 0, 2, 3)).astype(np.float16)
    # chunk-0 carry-correction tables: r^(s+1) * {cos,sin}(theta*s)
    pc = rpw[:, 0, :] * np.cos(ang[:, :MC])
    qc = rpw[:, 0, :] * np.sin(ang[:, :MC])
    pc3 = blk(pc, 2).astype(np.float16)
    qc3 = blk(qc, 2).astype(np.float16)
    rb = np.broadcast_to(r32[:, None], (N, MC)).astype(np.float32)
    rb3 = blk(rb, 2).astype(np.float32)
    ph48 = th * float(LLOC)
    rot48 = np.stack(
        [np.cos(ph48), np.sin(ph48), -np.sin(ph48)], axis=1)  # [N, 3]
    r48_3 = np.ascontiguousarray(
        rot48.reshape(2, 128, 3).transpose(1, 0, 2)).astype(np.float32)
    zrot = np.zeros_like(r48_3)

    # skip-path weights: hi/lo fp8 split of D, each as duplicated diag blocks
    # (both DoubleRow subtiles hold the same diag so the matmul contracts
    # D_i against x_hi + x_lo)
    d_hi = Dv.astype(f8).astype(np.float32)
    d_lo = (Dv - d_hi).astype(f8).astype(np.float32)
    dd5 = np.zeros((128, 2, 8, 2, 128), np.float32)
    idx = np.arange(128)
    for ki in range(8):
        for dup in range(2):
            dd5[idx, 0, ki, dup, idx] = d_hi[128 * ki + idx]
            dd5[idx, 1, ki, dup, idx] = d_lo[128 * ki + idx]
    dd5 = dd5.astype(f8)

    in_maps = []
    for c in range(NCORE):
        b, h = c // 2, c % 2
        xs = x[b, h * LLOC:(h + 1) * LLOC, :]                     # [LLOC, D]
        xT3 = np.ascontiguousarray(
            xs.T.reshape(8, 128, LLOC).transpose(1, 0, 2))        # [128,8,LLOC]
        x_hi = xT3.astype(f8)
        x_lo = (xT3 - x_hi.astype(np.float32)).astype(f8)
        xq4 = np.ascontiguousarray(
            np.stack([x_hi, x_lo], axis=1))                       # [128,2,8,LLOC]
        gm = np.zeros((128, 4), np.float32)
        pm = np.zeros((128, 4), np.float32)
        if h == 0:
            gm[:, b] = 1.0      # first-half core contributes to its pair's group
        pm[:, b] = 1.0          # every core selects its pair's group
        in_maps.append({
            "xq": xq4, "bg8": bg83, "ct": ct3,
            "cost": cs3, "sint": sn3, "rb": rb3, "rpow": rpw3,
            "pctab": pc3, "qctab": qc3,
            "rot48": (r48_3 if h == 1 else zrot),
            "gmask": gm, "pmask": pm, "ddiag": dd5,
        })
    return in_maps


def kernel(**inputs) -> np.ndarray:
    global LAST_RESULTS
    from concourse.bass_utils import run_bass_kernel_spmd

    if "nc" not in _CACHE:
        _CACHE["nc"] = _build()
    nc = _CACHE["nc"]

    in_maps = _prep(inputs)
    trace = os.environ.get("LRU_TRACE", "0") == "1"
    res = run_bass_kernel_spmd(
        nc, in_maps, core_ids=list(range(NCORE)), trace=trace,
        trace_cores=list(range(NCORE)) if trace else None,
        stitch_traces=trace,
    )
    LAST_RESULTS = res

    out = np.empty((B, L, D), np.float32)
    for c in range(NCORE):
        b, h = c // 2, c % 2
        o3 = np.asarray(res.results[c]["outT"])          # [128, 8, LLOC] f16
        o2 = o3.transpose(1, 0, 2).reshape(D, LLOC)      # [D, LLOC]
        out[b, h * LLOC:(h + 1) * LLOC, :] = o2.T.astype(np.float32)
    return out


# revision 55
# speedup vs baseline: 1.0214x; 1.0214x over previous
"""LRU forward on 8 Trainium2 NeuronCores.

Sharding: 8 shards = 4 batches x 2 sequence halves (L_local = 2048).
Per-core dataflow is fully transposed (d_model on SBUF partitions, time on
the free dim):

  input proj   Bu^T = Bg_cat^T @ x^T as fp8e4 DoubleRow matmuls (weights
               pre-scaled x64 on the host to stay in e4m3 normal range; the
               1/64 is folded into the PSUM->SBUF downcast scale)
  scan         complex diagonal recurrence -> rotating frame e^{-i theta t}
               turns it into 4 real per-lane scans (hardware
               tensor_tensor_scan; fp32 multiplier + internal state, fp16
               data).  Carry between sequence halves exchanged with a
               pairwise AllReduce and applied as g += r^{512m+s+1} * c_hat
               (decay table has the per-chunk factor folded in); chunk 0's
               rot-out runs carry-free inside the exchange bubble and is
               corrected afterwards via host P/Q tables (rpow x cos/sin)
  output proj  ys^T = CT_cat^T @ h in fp16, plus the D*x skip path as fp8
               DoubleRow matmuls over hi/lo split planes of x against
               duplicated diag(D_hi)/diag(D_lo) blocks (exact to ~0.4%)

Rotation elementwise work runs on DVE in fp16 (2x perf mode) with the c=1
imaginary path offloaded to gpsimd; PSUM->SBUF downcasts run on the scalar
(Activation) engine; tables are SBUF resident and loaded once.  Phase A uses
non-uniform time chunks (small at both ends) to shorten pipeline fill and
the scan->collective tail, and junk matmuls chained on the exchange arrival
keep the tensor engine's p-state ramped through the collective bubble.
Host side only preprocesses/shards and reassembles the output.
"""

import os

import numpy as np

B, L, D, N = 4, 4096, 1024, 256
NCORE = 8
LLOC = L // 2          # per-core sequence length
MC = 512               # time chunk (matmul moving free dim)
NMC = LLOC // MC       # 4 chunks
N2 = 2 * N             # stacked re|im channels

_CACHE = {}
LAST_RESULTS = None    # test.py reads exec_time_ns from here


def _build():
    import concourse.bass as bass
    import concourse.mybir as mybir
    import concourse.tile as tile
    from concourse import bacc

    f32 = mybir.dt.float32
    f16 = mybir.dt.float16
    f8 = mybir.dt.float8e4
    DR = mybir.MatmulPerfMode.DoubleRow
    ADD = mybir.AluOpType.add
    SUB = mybir.AluOpType.subtract
    MUL = mybir.AluOpType.mult

    nc = bacc.Bacc("TRN2", target_bir_lowering=False, debug=False, num_devices=NCORE)

    # ---- DRAM I/O (per-core) ----
    # xq holds fp8 hi/lo split planes of x^T: [:, 0] = fp8(x), [:, 1] =
    # fp8(x - hi).  The input projection reads the hi plane; the skip path
    # contracts both planes against duplicated diag(D_hi)/diag(D_lo) blocks.
    xqd = nc.dram_tensor("xq", [128, 2, 8, LLOC], f8, kind="ExternalInput").ap()
    bg8d = nc.dram_tensor("bg8", [128, 8, N2], f8, kind="ExternalInput").ap()
    ctd = nc.dram_tensor("ct", [128, 4, D], f16, kind="ExternalInput").ap()
    ddd = nc.dram_tensor("ddiag", [128, 2, 8, 2, 128], f8, kind="ExternalInput").ap()
    csd = nc.dram_tensor("cost", [128, 2, LLOC], f16, kind="ExternalInput").ap()
    snd = nc.dram_tensor("sint", [128, 2, LLOC], f16, kind="ExternalInput").ap()
    rbd = nc.dram_tensor("rb", [128, 2, MC], f32, kind="ExternalInput").ap()
    rpwd = nc.dram_tensor("rpow", [128, 2, NMC, MC], f16, kind="ExternalInput").ap()
    pcd = nc.dram_tensor("pctab", [128, 2, MC], f16, kind="ExternalInput").ap()
    qcd = nc.dram_tensor("qctab", [128, 2, MC], f16, kind="ExternalInput").ap()
    r48d = nc.dram_tensor("rot48", [128, 2, 3], f32, kind="ExternalInput").ap()
    gmd = nc.dram_tensor("gmask", [128, 4], f32, kind="ExternalInput").ap()
    pmd = nc.dram_tensor("pmask", [128, 4], f32, kind="ExternalInput").ap()
    outd = nc.dram_tensor("outT", [128, 8, LLOC], f16, kind="ExternalOutput").ap()

    with tile.TileContext(nc) as tc:
        from contextlib import ExitStack

        with ExitStack() as st:
            cpool = st.enter_context(tc.tile_pool(name="consts", bufs=1))
            xpool = st.enter_context(tc.tile_pool(name="xt", bufs=1))
            gpool = st.enter_context(tc.tile_pool(name="g", bufs=1))
            bpool = st.enter_context(tc.tile_pool(name="bu", bufs=3))
            upool = st.enter_context(tc.tile_pool(name="u", bufs=3))
            hpool = st.enter_context(tc.tile_pool(name="h", bufs=3))
            opool = st.enter_context(tc.tile_pool(name="o", bufs=3))
            ps = st.enter_context(tc.tile_pool(name="ps", bufs=2, space="PSUM"))
            dram = st.enter_context(tc.tile_pool(name="dram", bufs=1, space="DRAM"))

            # ---- SBUF residents; DMA order is pipeline priority order ----
            bg8_sb = cpool.tile([128, 8, N2], f8, tag="bg8", name="bg8")
            xq_sb = xpool.tile([128, 2, 8, LLOC], f8, tag="xq", name="xq")
            cs_sb = cpool.tile([128, 2, LLOC], f16, tag="cs", name="cs")
            sn_sb = cpool.tile([128, 2, LLOC], f16, tag="sn", name="sn")
            rb_sb = cpool.tile([128, 2, MC], f32, tag="rb", name="rb")
            ct_sb = cpool.tile([128, 4, D], f16, tag="ct", name="ct")
            dd_sb = cpool.tile([128, 2, 8, 2, 128], f8, tag="dd", name="dd")
            rpw_sb = cpool.tile([128, 2, NMC, MC], f16, tag="rpw", name="rpw")
            pc_sb = cpool.tile([128, 2, MC], f16, tag="pc", name="pc")
            qc_sb = cpool.tile([128, 2, MC], f16, tag="qc", name="qc")
            r48_sb = cpool.tile([128, 2, 3], f32, tag="r48", name="r48")
            gm_sb = cpool.tile([128, 4], f32, tag="gm", name="gm")
            pm_sb = cpool.tile([128, 4], f32, tag="pm", name="pm")

            # phase-A chunking: small chunks at the ends to shorten pipeline
            # fill and the scan->collective tail latency
            CHA = [(0, 256), (256, 256), (512, 512), (1024, 512),
                   (1536, 384), (1920, 128)]

            nc.sync.dma_start(bg8_sb[:, 0:4, :], bg8d[:, 0:4, :])
            nc.sync.dma_start(xq_sb[:, 0, :, 0:256], xqd[:, 0, :, 0:256])
            nc.sync.dma_start(bg8_sb[:, 4:8, :], bg8d[:, 4:8, :])
            nc.sync.dma_start(xq_sb[:, 0, :, 256:MC], xqd[:, 0, :, 256:MC])
            nc.sync.dma_start(cs_sb[:, :, 0:2 * MC], csd[:, :, 0:2 * MC])
            nc.sync.dma_start(sn_sb[:, :, 0:2 * MC], snd[:, :, 0:2 * MC])
            nc.sync.dma_start(xq_sb[:, 0, :, MC:2 * MC], xqd[:, 0, :, MC:2 * MC])
            nc.sync.dma_start(rb_sb[:], rbd[:, :, :])
            nc.sync.dma_start(xq_sb[:, 0, :, 2 * MC:3 * MC],
                              xqd[:, 0, :, 2 * MC:3 * MC])
            nc.sync.dma_start(cs_sb[:, :, 2 * MC:], csd[:, :, 2 * MC:])
            nc.sync.dma_start(xq_sb[:, 0, :, 3 * MC:], xqd[:, 0, :, 3 * MC:])
            nc.sync.dma_start(sn_sb[:, :, 2 * MC:], snd[:, :, 2 * MC:])
            nc.sync.dma_start(ct_sb[:], ctd[:, :, :])
            nc.sync.dma_start(dd_sb[:], ddd[:, :, :, :, :])
            nc.sync.dma_start(rpw_sb[:], rpwd[:, :, :, :])
            nc.sync.dma_start(pc_sb[:], pcd[:, :, :])
            nc.sync.dma_start(qc_sb[:], qcd[:, :, :])
            nc.sync.dma_start(r48_sb[:], r48d[:, :, :])
            nc.sync.dma_start(gm_sb[:], gmd[:, :])
            nc.sync.dma_start(pm_sb[:], pmd[:, :])
            # x lo-plane loads late: its consumers (skip-path matmuls) run
            # inside the collective bubble, and an early load would displace
            # phase-A-critical transfers on the DMA engines
            nc.sync.dma_start(xq_sb[:, 1, :, :], xqd[:, 1, :, :])

            g4 = []
            for tt_ in range(4):
                g4.append(gpool.tile([128, LLOC], f16, tag=f"g{tt_}", name=f"g{tt_}"))


            # ---- phase A: input projection + rot-in + scan, per time chunk ----
            # input proj: fp8 DoubleRow matmuls (weights pre-scaled x64 on the
            # host; the 1/64 is folded into the PSUM->SBUF downcast scale).
            # rot-in: the c=1 imaginary path runs on gpsimd to unload DVE.
            for m, (st, w) in enumerate(CHA):
                ms = slice(st, st + w)
                bu = []
                for j in range(4):
                    pt = ps.tile([128, MC], f32, tag=f"p{j}", name=f"pbu{j}_{m}")
                    for k2 in range(4):
                        nc.tensor.matmul(
                            pt[:, 0:w],
                            bg8_sb[:, 2 * k2:2 * k2 + 2, 128 * j:128 * (j + 1)],
                            xq_sb[:, 0, 2 * k2:2 * k2 + 2, ms],
                            start=(k2 == 0),
                            stop=(k2 == 3),
                            perf_mode=DR,
                        )
                    bt = bpool.tile([128, MC], f16, tag=f"bu{j}", name=f"bu{j}_{m}")
                    nc.scalar.mul(bt[:, 0:w], pt[:, 0:w], 1.0 / 64.0)
                    bu.append(bt)
                # rot-in, ordered by bu-copy availability (bu0, bu1, bu2, bu3
                # land in sequence from the Activation queue): ops consuming
                # early copies are emitted first so DVE never waits on a
                # later copy while an earlier one is ready
                u4 = [None] * 4
                cs0, sn0 = cs_sb[:, 0, ms], sn_sb[:, 0, ms]
                cs1, sn1 = cs_sb[:, 1, ms], sn_sb[:, 1, ms]
                t1a = upool.tile([128, MC], f16, tag="tmpA", name=f"tA0_{m}")
                nc.vector.tensor_tensor(t1a[:, 0:w], bu[0][:, 0:w], cs0, MUL)
                t4a = upool.tile([128, MC], f16, tag="tmpD0", name=f"tD0_{m}")
                nc.vector.tensor_tensor(t4a[:, 0:w], bu[0][:, 0:w], sn0, MUL)
                t1b = upool.tile([128, MC], f16, tag="tmpB0", name=f"tB0_{m}")
                nc.vector.tensor_tensor(t1b[:, 0:w], bu[1][:, 0:w], cs1, MUL)
                t4b = upool.tile([128, MC], f16, tag="tmpD1", name=f"tD1_{m}")
                nc.gpsimd.tensor_tensor(t4b[:, 0:w], bu[1][:, 0:w], sn1, MUL)
                t2a = upool.tile([128, MC], f16, tag="tmpC0", name=f"tC0_{m}")
                nc.vector.tensor_tensor(t2a[:, 0:w], bu[2][:, 0:w], sn0, MUL)
                t3a = upool.tile([128, MC], f16, tag="tmpE0", name=f"tE0_{m}")
                nc.vector.tensor_tensor(t3a[:, 0:w], bu[2][:, 0:w], cs0, MUL)
                u_re0 = upool.tile([128, MC], f16, tag="u0", name=f"u0_{m}")
                nc.vector.tensor_tensor(u_re0[:, 0:w], t1a[:, 0:w], t2a[:, 0:w], ADD)
                u_im0 = upool.tile([128, MC], f16, tag="u2", name=f"u2_{m}")
                nc.vector.tensor_tensor(u_im0[:, 0:w], t3a[:, 0:w], t4a[:, 0:w], SUB)
                u4[0], u4[2] = u_re0, u_im0
                t2b = upool.tile([128, MC], f16, tag="tmpC1", name=f"tC1_{m}")
                nc.vector.tensor_tensor(t2b[:, 0:w], bu[3][:, 0:w], sn1, MUL)
                t3b = upool.tile([128, MC], f16, tag="tmpE1", name=f"tE1_{m}")
                nc.gpsimd.tensor_tensor(t3b[:, 0:w], bu[3][:, 0:w], cs1, MUL)
                u_re1 = upool.tile([128, MC], f16, tag="u1", name=f"u1_{m}")
                nc.vector.tensor_tensor(u_re1[:, 0:w], t1b[:, 0:w], t2b[:, 0:w], ADD)
                u_im1 = upool.tile([128, MC], f16, tag="u3", name=f"u3_{m}")
                nc.gpsimd.tensor_tensor(u_im1[:, 0:w], t3b[:, 0:w], t4b[:, 0:w], SUB)
                u4[1], u4[3] = u_re1, u_im1
                for tt_ in range(4):
                    init = 0.0 if m == 0 else g4[tt_][:, st - 1:st]
                    nc.vector.tensor_tensor_scan(
                        g4[tt_][:, ms], rb_sb[:, tt_ & 1, 0:w], u4[tt_][:, 0:w],
                        init, MUL, ADD)

            # ---- phase B: carry exchange (pairwise AllReduce) ----
            stage = cpool.tile([128, 4], f32, tag="stage", name="stage")
            for tt_ in range(4):
                nc.vector.tensor_copy(stage[:, tt_:tt_ + 1], g4[tt_][:, LLOC - 1:LLOC])
            # scatter my carry into my pair's 4-column group (zero elsewhere)
            stage16 = cpool.tile([128, 16], f32, tag="stage16", name="stage16")
            for p in range(4):
                nc.vector.tensor_scalar_mul(
                    stage16[:, 4 * p:4 * (p + 1)], stage[:], gm_sb[:, p:p + 1])
            in_cc = dram.tile([128, 16], f32, tag="incc", name="incc")
            out_cc = dram.tile([128, 16], f32, tag="outcc", name="outcc",
                               addr_space="Shared")
            nc.sync.dma_start(in_cc[:], stage16[:])
            if os.environ.get("LRU_NOCC", "0") == "1":
                # collective-free variant for TimelineSim bottleneck analysis
                nc.sync.dma_start(out_cc[:], in_cc[:])
            else:
                nc.gpsimd.collective_compute(
                    "AllReduce",
                    mybir.AluOpType.add,
                    replica_groups=[list(range(NCORE))],
                    ins=[in_cc.opt()],
                    outs=[out_cc.opt()],
                )
            recv16 = cpool.tile([128, 16], f32, tag="recv16", name="recv16")
            nc.sync.dma_start(recv16[:], out_cc[:])


            # skip-path matmuls for chunk 0 run inside the collective bubble
            m0 = slice(0, MC)
            pre_ps = []
            with tc.high_priority():
                for di in range(8):
                    pt = ps.tile([128, MC], f32, tag=f"p{di % 4}", name=f"o{di}_0")
                    if di > 0:
                        for i in range(2):
                            nc.tensor.matmul(pt[:], dd_sb[:, i, di, :, :],
                                             xq_sb[:, :, di, m0],
                                             start=(i == 0), stop=False,
                                             perf_mode=DR)
                    pre_ps.append(pt)

            # PE warm-up chained on the exchange arrival: junk matmuls into a
            # corner of pre_ps[0] (its real skip matmul below uses start=True
            # and overwrites) keep the tensor engine's p-state ramped through
            # the bubble so the first output matmuls run at full clock
            jsrc = cpool.tile([128, 16], f16, tag="jsrc", name="jsrc")
            nc.scalar.copy(jsrc[:], recv16[:])
            for _ in range(60):
                nc.tensor.matmul(pre_ps[0][0:16, 0:128], jsrc[:, 0:16],
                                 cs_sb[:, 0, 0:128], start=True, stop=True)
            for i in range(2):
                nc.tensor.matmul(pre_ps[0][:], dd_sb[:, i, 0, :, :],
                                 xq_sb[:, :, 0, m0],
                                 start=(i == 0), stop=False, perf_mode=DR)

            # chunk-0 carry-free rot-out also runs inside the bubble; the
            # carry correction lands later via the P/Q tables (rpow folded
            # with cos/sin on the host) and per-lane chat scalars.
            # wait-hint: schedule these AFTER the exchange staging ops above
            # (their data deps are ready much earlier and the scheduler would
            # otherwise run them first, delaying the exchange).
            tc.tile_set_cur_wait(0.040)
            h4_0 = [None] * 4
            for c in range(2):
                csm = cs_sb[:, c, m0]
                snm = sn_sb[:, c, m0]
                t1 = hpool.tile([128, MC], f16, tag="tmpC", name=f"tE{c}_b")
                nc.vector.tensor_tensor(t1[:], g4[c][:, m0], csm, MUL)
                t2 = hpool.tile([128, MC], f16, tag="tmpD", name=f"tF{c}_b")
                nc.vector.tensor_tensor(t2[:], g4[2 + c][:, m0], snm, MUL)
                h_re = hpool.tile([128, MC], f16, tag=f"h{c}", name=f"h{c}_b")
                nc.vector.tensor_tensor(h_re[:], t1[:], t2[:], SUB)
                h4_0[c] = h_re
                t3 = hpool.tile([128, MC], f16, tag="tmpC", name=f"tG{c}_b")
                nc.vector.tensor_tensor(t3[:], g4[2 + c][:, m0], csm, MUL)
                t4 = hpool.tile([128, MC], f16, tag="tmpD", name=f"tH{c}_b")
                nc.vector.tensor_tensor(t4[:], g4[c][:, m0], snm, MUL)
                h_im = hpool.tile([128, MC], f16, tag=f"h{2+c}", name=f"h{2+c}_b")
                nc.vector.tensor_tensor(h_im[:], t3[:], t4[:], ADD)
                h4_0[2 + c] = h_im

            # select my pair's group: recv = sum_p recv16[:, 4p:4p+4] * pm[:, p]
            recv = cpool.tile([128, 4], f32, tag="recv", name="recv")
            nc.vector.tensor_scalar_mul(recv[:], recv16[:, 0:4], pm_sb[:, 0:1])
            for p in range(1, 4):
                nc.vector.scalar_tensor_tensor(
                    recv[:], recv16[:, 4 * p:4 * (p + 1)], pm_sb[:, p:p + 1],
                    recv[:], MUL, ADD)

            # chat = rot48 * recv (per-lane complex rotation), via STT pairs;
            # cols 4,5 hold -chat_im for the chunk-0 P/Q correction
            chat = cpool.tile([128, 6], f32, tag="chat", name="chat")
            tca = cpool.tile([128, 1], f32, tag="tca", name="tca")
            tcb = cpool.tile([128, 1], f32, tag="tcb", name="tcb")
            for c in range(2):
                c48 = r48_sb[:, c, 0:1]
                s48 = r48_sb[:, c, 1:2]
                ns48 = r48_sb[:, c, 2:3]
                # chat_re = recv_re*cos48 - recv_im*sin48
                nc.vector.tensor_tensor(tca[:], recv[:, c:c + 1], c48, MUL)
                nc.vector.scalar_tensor_tensor(
                    chat[:, c:c + 1], recv[:, 2 + c:3 + c], ns48, tca[:], MUL, ADD)
                # chat_im = recv_im*cos48 + recv_re*sin48
                nc.vector.tensor_tensor(tcb[:], recv[:, 2 + c:3 + c], c48, MUL)
                nc.vector.scalar_tensor_tensor(
                    chat[:, 2 + c:3 + c], recv[:, c:c + 1], s48, tcb[:], MUL, ADD)
                nc.vector.tensor_scalar_mul(
                    chat[:, 4 + c:5 + c], chat[:, 2 + c:3 + c], -1.0)

            # ---- phase C: carry fix + rot-out + output projection ----
            # h4 index: 0 = re(c0), 1 = re(c1), 2 = im(c0), 3 = im(c1); the
            # output matmuls consume them in that order, so compute the re
            # components (and their carry fixes) first.
            # tile_wait_until: scheduler-only hint that phase C becomes ready
            # late (after the carry exchange) so ready-early work (the
            # skip-path matmuls) is ordered ahead of it in the engine streams.
            tc.tile_set_cur_wait(0.045)
            for m in range(NMC):
                ms = slice(m * MC, (m + 1) * MC)

                def fix(mm, tt_):
                    msf = slice(mm * MC, (mm + 1) * MC)
                    nc.vector.scalar_tensor_tensor(
                        g4[tt_][:, msf],
                        rpw_sb[:, tt_ & 1, mm, :],
                        chat[:, tt_:tt_ + 1],
                        g4[tt_][:, msf],
                        MUL,
                        ADD,
                    )

                if m == 0:
                    # apply the carry to the pre-computed local rot-out:
                    # h_re += chat_re*P - chat_im*Q ; h_im += chat_im*P + chat_re*Q
                    # chunk 1's carry fixes are interleaved between the
                    # correction pairs so its rot-out ladder starts before
                    # all of chunk 0's corrections retire on DVE
                    h4 = h4_0
                    for comp, pcol, qcol, pre in ((0, 0, 4, 0), (1, 1, 5, 2),
                                                  (2, 2, 0, 1), (3, 3, 1, 3)):
                        c = comp & 1
                        nc.vector.scalar_tensor_tensor(
                            h4[comp][:], pc_sb[:, c, :], chat[:, pcol:pcol + 1],
                            h4[comp][:], MUL, ADD)
                        nc.vector.scalar_tensor_tensor(
                            h4[comp][:], qc_sb[:, c, :], chat[:, qcol:qcol + 1],
                            h4[comp][:], MUL, ADD)
                        fix(1, pre)
                else:
                    # carry fix (STT only runs on DVE; the compiler rejects
                    # it on the gpsimd engine); chunk 1's were issued above
                    if m > 1:
                        for tt_ in (0, 2, 1, 3):
                            fix(m, tt_)
                    h4 = [None] * 4
                    for c in range(2):
                        csm = cs_sb[:, c, ms]
                        snm = sn_sb[:, c, ms]
                        t1 = hpool.tile([128, MC], f16, tag="tmpC", name=f"tE{c}_{m}")
                        nc.vector.tensor_tensor(t1[:], g4[c][:, ms], csm, MUL)
                        t2 = hpool.tile([128, MC], f16, tag="tmpD", name=f"tF{c}_{m}")
                        nc.vector.tensor_tensor(t2[:], g4[2 + c][:, ms], snm, MUL)
                        h_re = hpool.tile([128, MC], f16, tag=f"h{c}", name=f"h{c}_{m}")
                        nc.vector.tensor_tensor(h_re[:], t1[:], t2[:], SUB)
                        h4[c] = h_re
                    for c in range(2):
                        csm = cs_sb[:, c, ms]
                        snm = sn_sb[:, c, ms]
                        t3 = hpool.tile([128, MC], f16, tag="tmpC", name=f"tG{c}_{m}")
                        nc.vector.tensor_tensor(t3[:], g4[2 + c][:, ms], csm, MUL)
                        t4 = hpool.tile([128, MC], f16, tag="tmpD", name=f"tH{c}_{m}")
                        nc.vector.tensor_tensor(t4[:], g4[c][:, ms], snm, MUL)
                        h_im = hpool.tile([128, MC], f16, tag=f"h{2+c}",
                                          name=f"h{2+c}_{m}")
                        nc.vector.tensor_tensor(h_im[:], t3[:], t4[:], ADD)
                        h4[2 + c] = h_im
                ot = opool.tile([128, 8, MC], f16, tag="ot", name=f"ot_{m}")
                for di in range(8):
                    if m == 0:
                        pt = pre_ps[di]
                    else:
                        pt = ps.tile([128, MC], f32, tag=f"p{di % 4}",
                                     name=f"o{di}_{m}")
                        for i in range(2):
                            nc.tensor.matmul(pt[:], dd_sb[:, i, di, :, :],
                                             xq_sb[:, :, di, ms],
                                             start=(i == 0), stop=False,
                                             perf_mode=DR)
                    for tt_ in range(4):
                        nc.tensor.matmul(
                            pt[:],
                            ct_sb[:, tt_, 128 * di:128 * (di + 1)],
                            h4[tt_][:],
                            start=False,
                            stop=(tt_ == 3),
                        )
                    nc.scalar.copy(ot[:, di, :], pt[:])
                    if di == 3:
                        nc.sync.dma_start(outd[:, 0:4, ms], ot[:, 0:4, :])
                    elif m == NMC - 1 and di in (5, 6):
                        nc.sync.dma_start(outd[:, di - 1:di, ms],
                                          ot[:, di - 1:di, :])
                if m == NMC - 1:
                    nc.sync.dma_start(outd[:, 6:7, ms], ot[:, 6:7, :])
                    nc.sync.dma_start(outd[:, 7:8, ms], ot[:, 7:8, :])
                else:
                    nc.sync.dma_start(outd[:, 4:8, ms], ot[:, 4:8, :])

    nc.compile()
    return nc


def _prep(inputs):
    """Host-side parameter prep + sharding. Returns per-core input maps."""
    x = np.asarray(inputs["input_sequence"], np.float32)
    nu_log = np.asarray(inputs["nu_log"], np.float32)
    theta_log = np.asarray(inputs["theta_log"], np.float32)
    B_re = np.asarray(inputs["B_re"], np.float32)
    B_im = np.asarray(inputs["B_im"], np.float32)
    C_re = np.asarray(inputs["C_re"], np.float32)
    C_im = np.asarray(inputs["C_im"], np.float32)
    Dv = np.asarray(inputs["D"], np.float32)

    r32 = np.exp(-np.exp(nu_log, dtype=np.float32), dtype=np.float32)
    th = np.exp(theta_log, dtype=np.float32).astype(np.float64)
    gamma = np.sqrt((1.0 - r32 * r32).astype(np.float32))

    def blk(a, nb):  # [nb*128, F] -> [128, nb, F]
        return np.ascontiguousarray(
            a.reshape(nb, 128, a.shape[-1]).transpose(1, 0, 2))

    import ml_dtypes
    f8 = ml_dtypes.float8_e4m3

    bg = np.concatenate(
        [(gamma[:, None] * B_re).T, (gamma[:, None] * B_im).T], axis=1)  # [D, 512]
    ct = np.concatenate([C_re.T, -C_im.T], axis=0)                      # [512, D]
    # fp8 weights pre-scaled x64 to stay in e4m3 normal range; the matmul
    # consumer divides by 64 during the PSUM downcast
    bg83 = blk(bg * 64.0, 8).astype(f8)
    ct3 = blk(ct, 4).astype(np.float16)

    t = np.arange(LLOC, dtype=np.float64)
    ang = th[:, None] * t[None, :]
    cs3 = blk(np.cos(ang), 2).astype(np.float16)
    sn3 = blk(np.sin(ang), 2).astype(np.float16)
    r64 = r32.astype(np.float64)
    s = np.arange(MC, dtype=np.float64)
    # r^(512m + s + 1), chunk factor folded in
    rpw = (r64[:, None, None]
           ** (MC * np.arange(NMC, dtype=np.float64)[None, :, None]
               + s[None, None, :] + 1.0))                     # [N, NMC, MC]
    rpw3 = np.ascontiguousarray(
        rpw.reshape(2, 128, NMC, MC).transpose(1, 0, 2, 3)).astype(np.float16)
    # chunk-0 carry-correction tables: r^(s+1) * {cos,sin}(theta*s)
    pc = rpw[:, 0, :] * np.cos(ang[:, :MC])
    qc = rpw[:, 0, :] * np.sin(ang[:, :MC])
    pc3 = blk(pc, 2).astype(np.float16)
    qc3 = blk(qc, 2).astype(np.float16)
    rb = np.broadcast_to(r32[:, None], (N, MC)).astype(np.float32)
    rb3 = blk(rb, 2).astype(np.float32)
    ph48 = th * float(LLOC)
    rot48 = np.stack(
        [np.cos(ph48), np.sin(ph48), -np.sin(ph48)], axis=1)  # [N, 3]
    r48_3 = np.ascontiguousarray(
        rot48.reshape(2, 128, 3).transpose(1, 0, 2)).astype(np.float32)
    zrot = np.zeros_like(r48_3)

    # skip-path weights: hi/lo fp8 split of D, each as duplicated diag blocks
    # (both DoubleRow subtiles hold the same diag so the matmul contracts
    # D_i against x_hi + x_lo)
    d_hi = Dv.astype(f8).astype(np.float32)
    d_lo = (Dv - d_hi).astype(f8).astype(np.float32)
    dd5 = np.zeros((128, 2, 8, 2, 128), np.float32)
    idx = np.arange(128)
    for ki in range(8):
        for dup in range(2):
            dd5[idx, 0, ki, dup, idx] = d_hi[128 * ki + idx]
            dd5[idx, 1, ki, dup, idx] = d_lo[128 * ki + idx]
    dd5 = dd5.astype(f8)

    in_maps = []
    for c in range(NCORE):
        b, h = c // 2, c % 2
        xs = x[b, h * LLOC:(h + 1) * LLOC, :]                     # [LLOC, D]
        xT3 = np.ascontiguousarray(
            xs.T.reshape(8, 128, LLOC).transpose(1, 0, 2))        # [128,8,LLOC]
        x_hi = xT3.astype(f8)
        x_lo = (xT3 - x_hi.astype(np.float32)).astype(f8)
        xq4 = np.ascontiguousarray(
            np.stack([x_hi, x_lo], axis=1))                       # [128,2,8,LLOC]
        gm = np.zeros((128, 4), np.float32)
        pm = np.zeros((128, 4), np.float32)
        if h == 0:
            gm[:, b] = 1.0      # first-half core contributes to its pair's group
        pm[:, b] = 1.0          # every core selects its pair's group
        in_maps.append({
            "xq": xq4, "bg8": bg83, "ct": ct3,
            "cost": cs3, "sint": sn3, "rb": rb3, "rpow": rpw3,
            "pctab": pc3, "qctab": qc3,
            "rot48": (r48_3 if h == 1 else zrot),
            "gmask": gm, "pmask": pm, "ddiag": dd5,
        })
    return in_maps


def kernel(**inputs) -> np.ndarray:
    global LAST_RESULTS
    from concourse.bass_utils import run_bass_kernel_spmd

    if "nc" not in _CACHE:
        _CACHE["nc"] = _build()
    nc = _CACHE["nc"]

    in_maps = _prep(inputs)
    trace = os.environ.get("LRU_TRACE", "0") == "1"
    res = run_bass_kernel_spmd(
        nc, in_maps, core_ids=list(range(NCORE)), trace=trace,
        trace_cores=list(range(NCORE)) if trace else None,
        stitch_traces=trace,
    )
    LAST_RESULTS = res

    out = np.empty((B, L, D), np.float32)
    for c in range(NCORE):
        b, h = c // 2, c % 2
        o3 = np.asarray(res.results[c]["outT"])          # [128, 8, LLOC] f16
        o2 = o3.transpose(1, 0, 2).reshape(D, LLOC)      # [D, LLOC]
        out[b, h * LLOC:(h + 1) * LLOC, :] = o2.T.astype(np.float32)
    return out
